# revision 32
# baseline (speedup 1.0000x reference)
"""Trainium2 Bass kernel for nn_Diffusion_65335042506822 (gnn_message_passing).

Strategy (8 NeuronCores, data-parallel over source residues):
  - Each core owns 128 source residues -> 6144 edges (128 res x 48 nbrs).
  - Host does index gathers (pair rows, neighbour pos/frames) and the tiny
    O(N) per-node work (backbone frames, local projections), then builds the
    per-edge geometric feature matrix in transposed (feature, edge) layout.
  - Device does the heavy per-edge compute: the 640x128 feature contraction
    (LayerNorm-centering folded into the weights), variance/rstd, and the
    128->256->128 gelu MLP.  All matmuls run as float32r (full PE rate).
  - LayerNorm trick: the mean-centering matrix M_c = I - 11^T/128 is folded
    into every weight chunk on the host, so the PSUM accumulation directly
    produces the centered pre-LN activations and the device only needs
    rstd = exp(-0.5*ln(mean(x^2)+eps)) (Ln+Exp live in the same ACT table
    set as Exp; Sqrt does not).
  - ACT table sets: phase A uses natural_log_exp_and_others (Square/Ln/Exp),
    phase B uses gelu_apprx_tanh_and_others (matches jax.nn.gelu default
    approximate=True).  A barrier separates them so the scheduler cannot
    interleave and thrash the 2.7us table loads.
"""

import sys
import numpy as np

for _p in ("/opt/trn_rl_repo",):
    if _p not in sys.path:
        sys.path.insert(0, _p)

N, K, A, LOCAL, PAIR = 1024, 48, 5, 512, 128
RBF_BINS = 16
NCORES = 8
NLOC = N // NCORES          # 128 residues per core
E = NLOC * K                # 6144 edges per core
EC = 1024                   # edge chunk (free dim) for compute ops
NEC = E // EC               # 6
ES = 512                    # matmul moving sub-chunk (one PSUM bank)
D_DIR, D_ROT, D_VEC, D_RBF = 75, 9, 30, 400
# feature row layout: [dir 75 | rot 9 | vec 30 | pad 14 | rbf 400 | pad 112]
OFF_DIR, OFF_ROT, OFF_VEC = 0, 75, 84
OFF_RBF = 128
FTOT = 640                  # 5 chunks of 128
NKCH = 6                    # 5 feature chunks + 1 p0 (identity/M_c) chunk

DEVICE_RBF = True           # compute the 400 RBF features on-device
D_HOST = D_DIR + D_ROT + D_VEC        # 114 host-built feature rows
RBF_STEP = 22.0 / RBF_BINS
RBF_CENTERS = np.linspace(0.0, 22.0, RBF_BINS, dtype=np.float32)

_CACHE = {}


def _geom_mats():
    """Constant selector/coefficient matrices for on-device RBF."""
    f4 = np.float32
    moff = np.zeros((30, 75), dtype=f4)
    for a in range(A):
        for b in range(A):
            for c in range(3):
                col = (a * A + b) * 3 + c
                moff[a * 3 + c, col] = 1.0
                moff[15 + b * 3 + c, col] = -1.0
    dsum = np.zeros((75, 25), dtype=f4)
    for ab in range(25):
        for c in range(3):
            dsum[ab * 3 + c, ab] = 1.0
    s2 = RBF_STEP * RBF_STEP
    # stacked rhs layout (32-aligned blocks): [u^2 @0, u @32] with u = d - 11.
    # Centering keeps matmul product magnitudes small where the gaussian is
    # non-negligible, which is what bounds the f32r product-rounding error.
    c2k = np.zeros((PAIR, 512), dtype=f4)
    brbf = np.zeros((PAIR, 4), dtype=f4)
    for g in range(D_RBF):
        ab, bin_ = g // RBF_BINS, g % RBF_BINS
        cm = float(RBF_CENTERS[bin_]) - 11.0
        c2k[ab, g] = -1.0 / s2
        c2k[32 + ab, g] = 2.0 * cm / s2
        brbf[g % PAIR, g // PAIR] = -(cm * cm) / s2
    return moff, dsum, c2k, brbf


def _frames_np(pos):
    def norm(v):
        return v / np.sqrt(np.sum(v * v, axis=-1, keepdims=True) + 1e-8)
    n, ca, c = pos[:, 0], pos[:, 1], pos[:, 2]
    e1 = norm(c - ca)
    v2 = n - ca
    e2 = norm(v2 - np.sum(v2 * e1, -1, keepdims=True) * e1)
    e3 = np.cross(e1, e2)
    R = np.stack([e1, e2, e3], axis=-1).astype(np.float32)  # (N,3,3)
    return R, ca.astype(np.float32)


def _host_prep(local, pos, pair, neighbours, mask,
               W_li, W_lj, W_dist, W_dir, W_rot, W_vec,
               ln_scale, ln_bias, mlp_w1, mlp_b1, mlp_w2, mlp_b2):
    f4 = np.float32
    nb = neighbours.astype(np.int64)
    R, t = _frames_np(pos.astype(f4))
    li = (local @ W_li).astype(f4)
    lj = (local @ W_lj).astype(f4)

    pos_nb = pos[nb]                      # (N,K,A,3)
    R_j = R[nb]                           # (N,K,3,3)
    off = pos[:, None, :, None, :] - pos_nb[:, :, None, :, :]   # (N,K,A,A,3)
    if not DEVICE_RBF:
        dist = np.sqrt(np.sum(off * off, -1) + 1e-8)            # (N,K,A,A)
        centers = np.linspace(0.0, 22.0, RBF_BINS, dtype=f4)
        step = f4(22.0 / RBF_BINS)
        rbf = np.exp(-(((dist[..., None] - centers) / step) ** 2)).astype(f4)
        rbf = rbf.reshape(N, K, D_RBF)

    off_local = np.einsum('nij,nkabi->nkabj', R, off).astype(f4)
    nrm = np.sqrt(np.sum(off_local * off_local, -1, keepdims=True) + 1e-8)
    dirf = (off_local / nrm).reshape(N, K, D_DIR).astype(f4)

    rot = np.einsum('nai,nkaj->nkij', R, R_j).reshape(N, K, D_ROT).astype(f4)

    xs = np.einsum('nij,nai->naj', R, pos - t[:, None])          # (N,A,3)
    xn = np.einsum('nij,nkai->nkaj', R, pos_nb - t[:, None, None])
    vec = np.concatenate(
        (0.1 * np.broadcast_to(xs[:, None], xn.shape), 0.1 * xn),
        axis=-1).reshape(N, K, D_VEC).astype(f4)

    p0 = (pair[np.arange(N)[:, None], nb] + li[:, None] + lj[nb]).astype(f4)

    nfeat = D_HOST if DEVICE_RBF else FTOT
    feats = np.zeros((N, K, nfeat), dtype=f4)
    feats[..., OFF_DIR:OFF_DIR + D_DIR] = dirf
    feats[..., OFF_ROT:OFF_ROT + D_ROT] = rot
    feats[..., OFF_VEC:OFF_VEC + D_VEC] = vec
    if not DEVICE_RBF:
        feats[..., OFF_RBF:OFF_RBF + D_RBF] = rbf

    # ---- weights, with LayerNorm centering folded in ----
    Mc = (np.eye(PAIR, dtype=f4) - 1.0 / PAIR).astype(f4)
    Wcat = np.zeros((FTOT, PAIR), dtype=f4)
    Wcat[OFF_DIR:OFF_DIR + D_DIR] = W_dir
    Wcat[OFF_ROT:OFF_ROT + D_ROT] = W_rot
    Wcat[OFF_VEC:OFF_VEC + D_VEC] = W_vec
    Wcat[OFF_RBF:OFF_RBF + D_RBF] = W_dist
    wc = np.zeros((PAIR, NKCH * PAIR), dtype=f4)
    for k in range(5):
        wc[:, k * PAIR:(k + 1) * PAIR] = Wcat[k * PAIR:(k + 1) * PAIR] @ Mc
    wc[:, 5 * PAIR:6 * PAIR] = Mc

    W1p = (ln_scale[:, None] * mlp_w1).astype(f4)           # (128,256)
    b1p = (ln_bias @ mlp_w1 + mlp_b1).astype(f4)            # (256,)
    b1 = np.ascontiguousarray(b1p.reshape(2, PAIR).T)       # (128,2)
    w2 = np.ascontiguousarray(
        np.concatenate([mlp_w2[0:PAIR], mlp_w2[PAIR:2 * PAIR]], axis=1))  # (128,256)
    b2 = np.ascontiguousarray(mlp_b2[:, None])              # (128,1)

    shared = dict(wc=wc, w1=np.ascontiguousarray(W1p), b1=b1, w2=w2, b2=b2)
    if DEVICE_RBF:
        moff, dsum, c2k, brbf = _geom_mats()
        shared.update(moff=moff, dsum=dsum, c2k=c2k, brbf=brbf)
        qi = (pos - t[:, None]).reshape(N, 1, A * 3).astype(f4)
        qi = np.broadcast_to(qi, (N, K, A * 3))
        qj = (pos_nb - t[:, None, None]).reshape(N, K, A * 3).astype(f4)
        qq = np.concatenate([qi, qj], axis=-1)      # (N,K,30)
    in_maps = []
    for c in range(NCORES):
        s = slice(c * NLOC, (c + 1) * NLOC)
        featsT = np.ascontiguousarray(feats[s].reshape(E, nfeat).T)
        p0T = np.ascontiguousarray(p0[s].reshape(E, PAIR).T)
        m = dict(featsT=featsT, p0T=p0T)
        if DEVICE_RBF:
            m["qqT"] = np.ascontiguousarray(qq[s].reshape(E, 30).T)
        m.update(shared)
        in_maps.append(m)

    pair_mask = (mask[:, None] & mask[nb]) & (neighbours != -1)
    return in_maps, pair_mask


def _build_nc():
    import concourse.tile as tile
    from concourse import bacc, mybir
    from concourse import bass_isa
    from contextlib import ExitStack

    f32 = mybir.dt.float32
    f32r = mybir.dt.float32r
    AF = mybir.ActivationFunctionType
    ALU = mybir.AluOpType

    nc = bacc.Bacc("TRN2", target_bir_lowering=False, debug=False)
    nfeat = D_HOST if DEVICE_RBF else FTOT
    featsT = nc.dram_tensor("featsT", [nfeat, E], f32r, kind="ExternalInput").ap()
    p0T = nc.dram_tensor("p0T", [PAIR, E], f32r, kind="ExternalInput").ap()
    if DEVICE_RBF:
        qqT = nc.dram_tensor("qqT", [30, E], f32r, kind="ExternalInput").ap()
        moff = nc.dram_tensor("moff", [30, 75], f32r, kind="ExternalInput").ap()
        dsum = nc.dram_tensor("dsum", [75, 25], f32r, kind="ExternalInput").ap()
        c2k = nc.dram_tensor("c2k", [PAIR, 512], f32r, kind="ExternalInput").ap()
        brbf = nc.dram_tensor("brbf", [PAIR, 4], f32, kind="ExternalInput").ap()
    wc = nc.dram_tensor("wc", [PAIR, NKCH * PAIR], f32r, kind="ExternalInput").ap()
    w1 = nc.dram_tensor("w1", [PAIR, 2 * PAIR], f32r, kind="ExternalInput").ap()
    b1 = nc.dram_tensor("b1", [PAIR, 2], f32, kind="ExternalInput").ap()
    w2 = nc.dram_tensor("w2", [PAIR, 2 * PAIR], f32r, kind="ExternalInput").ap()
    b2 = nc.dram_tensor("b2", [PAIR, 1], f32, kind="ExternalInput").ap()
    outT = nc.dram_tensor("outT", [PAIR, E], f32, kind="ExternalOutput").ap()

    with ExitStack() as ctx:
        tc = ctx.enter_context(tile.TileContext(nc))
        wp = ctx.enter_context(tc.tile_pool(name="w", bufs=1))
        fp = ctx.enter_context(tc.tile_pool(name="f", bufs=2))
        sp = ctx.enter_context(tc.tile_pool(name="s", bufs=2))
        pp = ctx.enter_context(tc.tile_pool(name="ph", bufs=1))


        wc_t = wp.tile([PAIR, NKCH * PAIR], f32r)
        nc.sync.dma_start(wc_t[:], wc[:])
        w1_t = wp.tile([PAIR, 2 * PAIR], f32r)
        nc.sync.dma_start(w1_t[:], w1[:])
        b1_t = wp.tile([PAIR, 2], f32)
        nc.sync.dma_start(b1_t[:], b1[:])
        w2_t = wp.tile([PAIR, 2 * PAIR], f32r)
        nc.sync.dma_start(w2_t[:], w2[:])
        b2_t = wp.tile([PAIR, 1], f32)
        nc.sync.dma_start(b2_t[:], b2[:])
        eps_t = wp.tile([PAIR, 1], f32)
        nc.vector.memset(eps_t[:], 1e-5)
        if DEVICE_RBF:
            moff_t = wp.tile([30, 75], f32r)
            nc.sync.dma_start(moff_t[:], moff[:])
            dsum_t = wp.tile([75, 25], f32r)
            nc.sync.dma_start(dsum_t[:], dsum[:])
            c2k_t = wp.tile([PAIR, 512], f32r)
            nc.sync.dma_start(c2k_t[:], c2k[:])
            stk_t = pp.tile([PAIR, E], f32r)
            nc.vector.memset(stk_t[:].bitcast(f32), 0.0)
            brbf_t = wp.tile([PAIR, 4], f32)
            nc.sync.dma_start(brbf_t[:], brbf[:])
            eps8_t = wp.tile([PAIR, 1], f32)
            nc.vector.memset(eps8_t[:], 1e-8)

        phat_t = pp.tile([PAIR, E], f32r)

        # ---------- phase A: feature contraction + standardization ----------
        with tc.tile_pool(name="psA", bufs=2, space="PSUM") as psA, \
             tc.tile_pool(name="psG", bufs=2, space="PSUM") as psG:
            for ec in range(NEC):
                lo = ec * EC
                if DEVICE_RBF:
                    f0 = fp.tile([D_HOST, EC], f32r, tag="f0")
                    nc.sync.dma_start(f0[:], featsT[:, lo:lo + EC])
                    qq = fp.tile([30, EC], f32r, tag="qq")
                    nc.sync.dma_start(qq[:], qqT[:, lo:lo + EC])
                    p0t = fp.tile([PAIR, EC], f32r, tag="p0")
                    nc.sync.dma_start(p0t[:], p0T[:, lo:lo + EC])

                    # off(ab,c) = q_i(a,c) - q_j(b,c)
                    offp = psG.tile([75, EC], f32, tag="g")
                    for es in range(EC // ES):
                        s2 = slice(es * ES, (es + 1) * ES)
                        nc.tensor.matmul(offp[:, s2], moff_t[:], qq[:, s2],
                                         start=True, stop=True)
                    sqs = sp.tile([75, EC], f32r, tag="sqs")
                    nc.scalar.activation(sqs[:], offp[:], AF.Square)
                    d2p = psG.tile([25, EC], f32, tag="g")
                    for es in range(EC // ES):
                        s2 = slice(es * ES, (es + 1) * ES)
                        nc.tensor.matmul(d2p[:, s2], dsum_t[:], sqs[:, s2],
                                         start=True, stop=True)
                    ld = sp.tile([25, EC], f32, tag="ld")
                    nc.scalar.activation(ld[:], d2p[:], AF.Ln,
                                         bias=eps8_t[0:25, :])
                    dstf = sp.tile([89, EC], f32, tag="dstf")
                    nc.scalar.activation(dstf[64:89, :], ld[:], AF.Exp, scale=0.5)
                    # centered basis u = d - 11, u^2 into stk
                    stks = stk_t[:, lo:lo + EC]
                    nc.vector.tensor_scalar_add(stks[32:57, :], dstf[64:89, :],
                                                -11.0)
                    nc.vector.tensor_mul(stks[0:25, :], stks[32:57, :],
                                         stks[32:57, :])
                    ftiles = [(f0, D_HOST)]
                    for k in range(4):
                        rows = PAIR if k < 3 else D_RBF - 3 * PAIR
                        u = psG.tile([PAIR, EC], f32, tag="g")
                        for es in range(EC // ES):
                            s2 = slice(es * ES, (es + 1) * ES)
                            nc.tensor.matmul(
                                u[0:rows, s2],
                                c2k_t[0:57, k * PAIR:k * PAIR + rows],
                                stks[0:57, s2], start=True, stop=True)
                        ft = sp.tile([rows, EC], f32r, tag=f"rbf{k}")
                        nc.scalar.activation(ft[:], u[0:rows, :], AF.Exp,
                                             bias=brbf_t[0:rows, k:k + 1])
                        ftiles.append((ft, rows))
                    ftiles.append((p0t, PAIR))
                else:
                    ftiles = []
                    for k in range(5):
                        ft = fp.tile([PAIR, EC], f32r, tag=f"f{k}")
                        nc.sync.dma_start(
                            ft[:], featsT[k * PAIR:(k + 1) * PAIR, lo:lo + EC])
                        ftiles.append((ft, PAIR))
                    p0t = fp.tile([PAIR, EC], f32r, tag="p0")
                    nc.sync.dma_start(p0t[:], p0T[:, lo:lo + EC])
                    ftiles.append((p0t, PAIR))

                pc = psA.tile([PAIR, EC], f32, tag="pc")
                for es in range(EC // ES):
                    s2 = slice(es * ES, (es + 1) * ES)
                    for k, (ft, kk) in enumerate(ftiles):
                        nc.tensor.matmul(
                            pc[:, s2],
                            wc_t[0:kk, k * PAIR:(k + 1) * PAIR],
                            ft[0:kk, s2],
                            start=(k == 0), stop=(k == NKCH - 1))

                psq = sp.tile([PAIR, EC], f32, tag="psq")
                nc.scalar.activation(psq[:], pc[:], AF.Square)
                m2b = sp.tile([PAIR, EC], f32, tag="m2b")
                nc.gpsimd.partition_all_reduce(m2b[:], psq[:], channels=PAIR,
                                               reduce_op=bass_isa.ReduceOp.add)
                lb = sp.tile([PAIR, EC], f32, tag="lb")
                nc.scalar.activation(lb[:], m2b[:], AF.Ln, scale=1.0 / PAIR,
                                     bias=eps_t[:])
                rstdb = sp.tile([PAIR, EC], f32, tag="rstdb")
                nc.scalar.activation(rstdb[:], lb[:], AF.Exp, scale=-0.5)
                nc.vector.tensor_mul(phat_t[:, lo:lo + EC], pc[:], rstdb[:])

        tc.strict_bb_all_engine_barrier()

        # ---------- phase B: MLP ----------
        with tc.tile_pool(name="psB", bufs=2, space="PSUM") as psB:
            for ec in range(NEC):
                lo = ec * EC
                h1 = psB.tile([PAIR, EC], f32, tag="h")
                h2 = psB.tile([PAIR, EC], f32, tag="h")
                for es in range(EC // ES):
                    s2 = slice(lo + es * ES, lo + (es + 1) * ES)
                    p2 = slice(es * ES, (es + 1) * ES)
                    nc.tensor.matmul(h1[:, p2], w1_t[:, 0:PAIR],
                                     phat_t[:, s2],
                                     start=True, stop=True)
                    nc.tensor.matmul(h2[:, p2], w1_t[:, PAIR:2 * PAIR],
                                     phat_t[:, s2],
                                     start=True, stop=True)
                g1 = sp.tile([PAIR, EC], f32r, tag="g1")
                nc.scalar.activation(g1[:], h1[:], AF.Gelu_apprx_tanh,
                                     bias=b1_t[:, 0:1])
                g2 = sp.tile([PAIR, EC], f32r, tag="g2")
                nc.scalar.activation(g2[:], h2[:], AF.Gelu_apprx_tanh,
                                     bias=b1_t[:, 1:2])
                op_ = psB.tile([PAIR, EC], f32, tag="op")
                for es in range(EC // ES):
                    p2 = slice(es * ES, (es + 1) * ES)
                    nc.tensor.matmul(op_[:, p2], w2_t[:, 0:PAIR],
                                     g1[:, p2],
                                     start=True, stop=False)
                    nc.tensor.matmul(op_[:, p2], w2_t[:, PAIR:2 * PAIR],
                                     g2[:, p2],
                                     start=False, stop=True)
                ot = sp.tile([PAIR, EC], f32, tag="ot")
                nc.vector.tensor_scalar_add(ot[:], op_[:], b2_t[:, 0:1])
                nc.sync.dma_start(outT[:, lo:lo + EC], ot[:])

    nc.compile()
    return nc


def _get_nc():
    if "nc" not in _CACHE:
        _CACHE["nc"] = _build_nc()
    return _CACHE["nc"]


def kernel(**inputs):
    from concourse.bass_utils import run_bass_kernel_spmd

    in_maps, pair_mask = _host_prep(**inputs)
    nc = _get_nc()
    res = run_bass_kernel_spmd(nc, in_maps, list(range(NCORES)))
    p = np.empty((N, K, PAIR), dtype=np.float32)
    for c in range(NCORES):
        outT = res.results[c]["outT"]          # (128, 6144)
        p[c * NLOC:(c + 1) * NLOC] = outT.T.reshape(NLOC, K, PAIR)
    return p, pair_mask
